# revision 1
# baseline (speedup 1.0000x reference)
"""Trainium2 Bass kernel for the sparse_attention nn module.

Sharding: 8 cores = 4 batches x 2 halves of the L=5120 attention rows.
Per core:
  - tiny projections (LSTM gates, K/V) run as partition-packed block-diagonal
    matmuls in bf16 ([t-blocks x channels] on partitions, n=256 free), with
    per-(t,channel) biases folded into ACT bias-copy instructions
  - K^T / V(k,d) / X^T layouts produced by SBUF->SBUF DMA relayouts
  - 2048->32 conv in bf16 from a single rearranged metadata DMA
  - bilinear grid-sample via one-hot outer-product weight matrix (DVE+GPSIMD
    split), PE transposes, lc = cm^T @ Wg^T matmuls
  - attention: scores^T = K @ Q^T (bf16, PE), sigmoid on ACT from PSUM,
    out^T accumulated as V^T @ probs^T in PSUM col-groups; ACT-bound and
    software-pipelined (ACT runs back-to-back).
"""
import sys

sys.path.insert(0, "/opt/trn_rl_repo")

import numpy as np

import concourse.bacc as bacc
import concourse.tile as tile
from concourse import mybir
from concourse.bass_utils import run_bass_kernel_spmd
from concourse.masks import make_identity

F32 = mybir.dt.float32
BF16 = mybir.dt.bfloat16
ALU = mybir.AluOpType
ACTF = mybir.ActivationFunctionType
F32R = mybir.dt.float32r

B, T, N = 4, 20, 256
L = T * N            # 5120
HL = L // 2          # 2560 rows per core
HT = T // 2          # 10 t-steps per core
CMAP, CC = 2048, 32
NK = L // 128        # 40 k-tiles
NQ = HL // 512       # 5 q-chunks
KG = 2               # k-tiles per sigmoid group

# csth (bf16 weight blob) column layout
C_KW, C_VW, C_GWI, C_GWO, C_GG = 0, 80, 160, 200, 240
C_VFX, C_VFLC, C_FC, C_FCO = 280, 284, 288, 296
CSTH_W = 304
# bia (f32 per-partition bias blob [80, 12]) column layout
B_KB0, B_KB1, B_VB0, B_VB1, B_GBI, B_GBO, B_GBG, B_FCB, B_VFB, B_FCOB, B_CMPB = (
    0, 1, 2, 3, 4, 5, 6, 7, 8, 9, 10)
BIA_W = 12

_nc_cache = None
DEBUG = False


def _build():
    nc = bacc.Bacc()
    dt_in = {
        "xpk": ([64, 256], F32R),     # global t: rows 2t+c (t0-9), 32+2t'+c (t10-19)
        "xg": ([20, 256], F32R),      # core's half t-local: rows 2tl+c
        "xpm": ([128, 20, 2], F32),   # point-major half coords
        "csth": ([64, CSTH_W], F32R),
        "bia": ([80, BIA_W], F32),
        "mdh": ([128, 16, 256], F32R),
        "cwh": ([128, 16, CC], F32R),
    }
    d = {k: nc.dram_tensor(k, sh, dt, kind="ExternalInput")
         for k, (sh, dt) in dt_in.items()}
    y_out = nc.dram_tensor("y", [2, HL], F32, kind="ExternalOutput")
    dr_X = nc.dram_tensor("dr_X", [40, 256], F32R, kind="Internal")
    dr_Kp = nc.dram_tensor("dr_Kp", [80, 2, 256], F32R, kind="Internal")
    dr_Vp = nc.dram_tensor("dr_Vp", [80, 2, 256], BF16, kind="Internal")

    with tile.TileContext(nc) as tc:
        with tc.tile_pool(name="main", bufs=1) as pool, \
             tc.tile_pool(name="work", bufs=3) as work, \
             tc.tile_pool(name="work2", bufs=2) as work2, \
             tc.tile_pool(name="ps", bufs=2, space="PSUM") as psp, \
             tc.tile_pool(name="pt", bufs=2, space="PSUM") as ptp, \
             tc.tile_pool(name="po", bufs=2, space="PSUM") as pop:

            # ---- input DMAs (one each, partition-parallel layouts) ----
            sb_xpm = pool.tile([128, 20, 2], F32)
            nc.sync.dma_start(sb_xpm, d["xpm"].ap())
            sb_csth = pool.tile([64, CSTH_W], F32R)
            nc.sync.dma_start(sb_csth, d["csth"].ap())
            sb_xg = pool.tile([20, 256], F32R)
            nc.sync.dma_start(sb_xg, d["xg"].ap())
            sb_bia = pool.tile([80, BIA_W], F32)
            nc.sync.dma_start(sb_bia, d["bia"].ap())
            sb_xpk = pool.tile([64, 256], F32R)
            nc.sync.dma_start(sb_xpk, d["xpk"].ap())
            sb_cwh = pool.tile([128, 16, CC], F32R)
            nc.sync.dma_start(sb_cwh, d["cwh"].ap())
            sb_mdh = pool.tile([128, 16, 256], F32R)
            for mp in range(4):
                nc.sync.dma_start(sb_mdh[:, 4 * mp:4 * mp + 4, :],
                                  d["mdh"].ap()[:, 4 * mp:4 * mp + 4, :])

            def W(c0, c1, r0=0, r1=20):
                return sb_csth[r0:r1, c0:c1]

            def bias(col, rows):
                return sb_bia[0:rows, col:col + 1]

            # 1-element dummy sigmoid pulls the ACT table load to t=0
            warm = pool.tile([1, 1], F32)
            nc.vector.memset(warm, 0.0)
            warm2 = pool.tile([1, 1], F32)
            nc.scalar.activation(warm2, warm, ACTF.Sigmoid)
            ident = pool.tile([128, 128], BF16)
            make_identity(nc, ident)
            identf = pool.tile([128, 128], F32)
            make_identity(nc, identf)
            iota16 = pool.tile([128, 16], F32)
            nc.gpsimd.iota(iota16, [[1, 16]], base=0, channel_multiplier=0,
                           allow_small_or_imprecise_dtypes=True)

            # ---- persistent SBUF results ----
            sb_KT = pool.tile([40, L], F32R)      # rows 0-7 K^T, 32-39 replica
            sb_QT = pool.tile([40, HL], F32R)
            sb_Vkd = pool.tile([128, 2, 2, HT, 8], BF16)  # (a, h, tl, d)
            sb_WgT = [pool.tile([128, HL], F32R, name=f"wgT{h}") for h in range(2)]
            sb_lc = pool.tile([CC, HL], F32R)
            sb_X2 = pool.tile([4, HL], F32R)
            sb_XT = pool.tile([4, HL], F32R)
            sb_cm = pool.tile([CC, 256], F32)
            sb_cmT = pool.tile([128, 2, CC], F32R)
            sb_Kp = pool.tile([80, 2, 256], F32R)
            sb_Vp = pool.tile([80, 2, 256], BF16)
            sb_y = pool.tile([2, HL], F32)

            # ---- LSTM gates (packed [4*tl + ch, n]) ----
            ps_i = ptp.tile([40, 256], F32, tag="tmp", name="ps_i")
            nc.tensor.matmul(ps_i, lhsT=W(C_GWI, C_GWO), rhs=sb_xg)
            ps_o2 = ptp.tile([40, 256], F32, tag="tmp", name="ps_o2")
            nc.tensor.matmul(ps_o2, lhsT=W(C_GWO, C_GG), rhs=sb_xg)
            Xi = work2.tile([40, 256], BF16, tag="Xi")
            nc.scalar.activation(Xi, ps_i, ACTF.Sigmoid, bias=bias(B_GBI, 40))
            Xo = work2.tile([40, 256], BF16, tag="Xo")
            nc.scalar.activation(Xo, ps_o2, ACTF.Sigmoid, bias=bias(B_GBO, 40))
            ps_g = ptp.tile([40, 256], F32, tag="tmp", name="ps_g")
            nc.tensor.matmul(ps_g, lhsT=W(C_GG, C_GG + 40), rhs=sb_xg)
            Xg = work2.tile([40, 256], BF16, tag="Xg")
            nc.scalar.activation(Xg, ps_g, ACTF.Tanh, bias=bias(B_GBG, 40))
            Xc = work2.tile([40, 256], BF16, tag="Xc")
            nc.vector.tensor_tensor(Xc, Xi, Xg, ALU.mult)
            # ---- K/V projections (packed, then DMA relayouts) ----
            for a in range(2):
                r0 = 32 * a
                rx = sb_xpk[r0:r0 + 20, :]
                ps_kp = ptp.tile([80, 256], F32, tag="tmp", name="ps_kp")
                nc.tensor.matmul(ps_kp, lhsT=W(C_KW, C_VW, r0, r0 + 20), rhs=rx)
                nc.scalar.activation(sb_Kp[:, a, :], ps_kp, ACTF.Identity,
                                     bias=bias(B_KB0 + a, 80))
                ps_vp = ptp.tile([80, 256], F32, tag="tmp", name="ps_vp")
                nc.tensor.matmul(ps_vp, lhsT=W(C_VW, C_GWI, r0, r0 + 20), rhs=rx)
                nc.scalar.activation(sb_Vp[:, a, :], ps_vp, ACTF.Identity,
                                     bias=bias(B_VB0 + a, 80))
            nc.sync.dma_start(dr_Kp.ap(), sb_Kp)
            nc.sync.dma_start(dr_Vp.ap(), sb_Vp)
            def emit_kt(a, r0, eng, t0=0, t1=HT):
                eng.dma_start(
                    sb_KT[r0:r0 + 8,
                          a * HL + t0 * 256:a * HL + t1 * 256].rearrange(
                        "c (t n) -> c t n", t=t1 - t0),
                    dr_Kp.ap()[8 * t0:8 * t1, a, :].rearrange(
                        "(t c) n -> c t n", c=8))

            Xc2 = work2.tile([40, 256], BF16, tag="Xc2")
            nc.scalar.activation(Xc2, Xc, ACTF.Tanh)
            sb_X = pool.tile([40, 256], F32R)
            nc.vector.tensor_tensor(sb_X, Xo, Xc2, ALU.mult)
            # relayout X packed -> X^T [4, HL] via DRAM scratch
            nc.sync.dma_start(dr_X.ap(), sb_X)

            def emit_xt(qc):
                nc.sync.dma_start(
                    sb_XT[:, qc * 512:(qc + 1) * 512].rearrange(
                        "c (t n) -> c t n", t=2),
                    dr_X.ap()[8 * qc:8 * qc + 8, :].rearrange(
                        "(t c) n -> c t n", c=4))
            emit_xt(0)
            emit_kt(0, 0, nc.scalar, 0, 5)
            for a in range(2):
                for h in range(2):
                    nc.sync.dma_start(
                        sb_Vkd[:, a, h, :, :],
                        dr_Vp.ap()[:, a, 128 * h:128 * (h + 1)].rearrange(
                            "(t c) p -> p t c", c=8))
            emit_xt(1)
            emit_kt(0, 0, nc.sync, 5, HT)
            emit_xt(2)
            emit_xt(3)
            emit_xt(4)


            # K^T band rows 0-7 and replica rows 32-39, per t-range half.
            # a=0 halves issue from ACT (idle pre-attention), a=1 from SP
            # inside the first attention chunk (needed only from group 10 on).


            # ---- compressed feature map cm then cmT ----
            ps_cm = ptp.tile([CC, 256], F32, tag="tmp", name="ps_cm")
            for k in range(16):
                nc.tensor.matmul(ps_cm, lhsT=sb_cwh[:, k, :],
                                 rhs=sb_mdh[:, k, :],
                                 start=(k == 0), stop=(k == 15))
            nc.scalar.activation(sb_cm, ps_cm, ACTF.Identity, bias=bias(B_CMPB, 32))
            for h in range(2):
                ps_ct = ptp.tile([128, CC], F32, tag="tmp", name="ps_ct")
                nc.tensor.transpose(ps_ct,
                                    sb_cm[:, h * 128:(h + 1) * 128],
                                    identf[0:CC, 0:CC])
                nc.vector.tensor_copy(sb_cmT[:, h, :], ps_ct)

            # ---- grid-sample weights (per-point scalars, big-tile DVE) ----
            ixy = pool.tile([128, 20, 2], F32)
            nc.vector.tensor_scalar(ixy, sb_xpm, 1.0 / 32.0, 0.5, ALU.mult, ALU.add)
            ti = pool.tile([128, 20, 2], mybir.dt.int32)
            nc.vector.tensor_copy(ti, ixy)
            tf = pool.tile([128, 20, 2], F32)
            nc.vector.tensor_copy(tf, ti)
            gt = pool.tile([128, 20, 2], F32)
            nc.vector.tensor_tensor(gt, tf, ixy, ALU.is_gt)
            x0f = pool.tile([128, 20, 2], F32)   # = floor coord + 1, in [0,16]
            nc.vector.tensor_tensor(x0f, tf, gt, ALU.subtract)
            fr = pool.tile([128, 20, 2], F32)
            nc.vector.tensor_tensor(fr, ixy, x0f, ALU.subtract)
            w0 = pool.tile([128, 20, 2], F32)
            nc.vector.tensor_scalar(w0, fr, -1.0, 1.0, ALU.mult, ALU.add)
            v0 = pool.tile([128, 20, 2], F32)
            nc.vector.tensor_scalar(v0, x0f, 0.5, None, ALU.is_ge)
            v1 = pool.tile([128, 20, 2], F32)
            nc.vector.tensor_scalar(v1, x0f, 15.5, None, ALU.is_le)
            w0e = pool.tile([128, 20, 2], F32)
            nc.vector.tensor_tensor(w0e, w0, v0, ALU.mult)
            w1e = pool.tile([128, 20, 2], F32)
            nc.vector.tensor_tensor(w1e, fr, v1, ALU.mult)
            x0c = pool.tile([128, 20, 2], F32)
            nc.vector.tensor_scalar(x0c, x0f, -1.0, 0.0, ALU.add, ALU.max)
            x1c = pool.tile([128, 20, 2], F32)
            nc.vector.tensor_scalar(x1c, x0f, 15.0, None, ALU.min)

            # one-hot corner weights Ox, Oy: [p, chunk, 16]; x-lane on DVE,
            # y-lane on GPSIMD so they build in parallel
            ohs = []
            oh_tmp = []
            for a in range(2):
                o_t = pool.tile([128, 20, 16], F32, name=f"oh{a}")
                tmp = pool.tile([128, 20, 16], F32, name=f"ohtmp{a}")
                ohs.append(o_t)
                oh_tmp.append(tmp)

            def emit_oh(c0, c1):
                n = c1 - c0
                sh = (128, n, 16)
                for a in range(2):
                    o_t = ohs[a][:, c0:c1, :]
                    tmp = oh_tmp[a][:, c0:c1, :]
                    nc.vector.tensor_tensor(
                        o_t, iota16[:, None, :].to_broadcast(sh),
                        x0c[:, c0:c1, a:a + 1].to_broadcast(sh), ALU.is_equal)
                    nc.vector.tensor_tensor(
                        o_t, o_t, w0e[:, c0:c1, a:a + 1].to_broadcast(sh),
                        ALU.mult)
                    nc.vector.tensor_tensor(
                        tmp, iota16[:, None, :].to_broadcast(sh),
                        x1c[:, c0:c1, a:a + 1].to_broadcast(sh), ALU.is_equal)
                    nc.vector.tensor_tensor(
                        tmp, tmp, w1e[:, c0:c1, a:a + 1].to_broadcast(sh),
                        ALU.mult)
                    nc.vector.tensor_tensor(o_t, o_t, tmp, ALU.add)
            emit_oh(0, 4)
            Ox, Oy = ohs

            # Wg[p, (py,px)] = Oy*Ox outer products; PE-transpose pairs of
            # 128-chunks into one PSUM tile, single DVE copy out per half.
            def emit_pair(pc, outer_eng, copy_eng=None, pspool=None):
                wgs = []
                for j in range(2):
                    c = 2 * pc + j
                    wg = work.tile([128, 16, 16], F32, tag=f"wg{j}",
                                   name=f"wg{j}")
                    outer_eng.tensor_tensor(
                        wg, Oy[:, c, :, None].to_broadcast((128, 16, 16)),
                        Ox[:, c, None, :].to_broadcast((128, 16, 16)),
                        ALU.mult)
                    wgs.append(wg.rearrange("p a b -> p (a b)"))
                for h in range(2):
                    if pspool is pop:
                        ps_p = pop.tile([128, 256], F32, tag="po", name="ps_p")
                    else:
                        ps_p = ptp.tile([128, 256], F32, tag="tmp", name="ps_p")
                    for j in range(2):
                        nc.tensor.transpose(
                            ps_p[:, j * 128:(j + 1) * 128],
                            wgs[j][:, h * 128:(h + 1) * 128],
                            identf)
                    nc.vector.tensor_copy(
                        sb_WgT[h][:, pc * 256:(pc + 1) * 256], ps_p)

            # local context lcT[ch, pts] -> X2 -> Q for one q-chunk, plus the
            # per-chunk Q^T replica DMA
            def emit_lcx2q(qc):
                qsl = slice(qc * 512, (qc + 1) * 512)
                ps_lc = ptp.tile([CC, 512], F32, tag="tmp", name="ps_lc")
                for h in range(2):
                    nc.tensor.matmul(
                        ps_lc, lhsT=sb_cmT[:, h, :],
                        rhs=sb_WgT[h][:, qsl],
                        start=(h == 0), stop=(h == 1))
                nc.vector.tensor_copy(sb_lc[:, qsl], ps_lc)
                ps_x2 = ptp.tile([4, 512], F32, tag="tmp", name="ps_x2")
                nc.tensor.matmul(ps_x2, lhsT=W(C_VFX, C_VFLC, 0, 4),
                                 rhs=sb_XT[:, qsl], start=True, stop=False)
                nc.tensor.matmul(ps_x2, lhsT=W(C_VFLC, C_FC, 0, 32),
                                 rhs=sb_lc[:, qsl], start=False, stop=True)
                nc.vector.tensor_scalar(sb_X2[:, qsl], ps_x2, bias(B_VFB, 4),
                                        None, ALU.add)
                ps_q = ptp.tile([8, 512], F32, tag="tmp", name="ps_q")
                nc.tensor.matmul(ps_q, lhsT=W(C_FC, C_FCO, 0, 4),
                                 rhs=sb_X2[:, qsl])
                nc.vector.tensor_scalar(sb_QT[0:8, qsl], ps_q, bias(B_FCB, 8),
                                        None, ALU.add)
                if qc > 0:
                    nc.sync.dma_start(sb_QT[32:40, qsl], sb_QT[0:8, qsl])

            # ---- attention (software-pipelined, ACT-bound) ----
            NG = NK // KG            # sigmoid groups per q-chunk
            TOT = NQ * NG
            ps_o = [None] * NQ
            ps_s_t = [None] * TOT

            def emit_scores(g):
                qc, kg = divmod(g, NG)
                if kg == 0:
                    ps_o[qc] = pop.tile([128, 512], F32, tag="po", name="ps_o")
                qsl = slice(qc * 512, (qc + 1) * 512)
                ps_s = psp.tile([128, KG * 512], F32, tag="scores", name="ps_s")
                for j in range(KG):
                    ki = kg * KG + j
                    rg = 32 * j if qc > 0 else 0
                    nc.tensor.matmul(
                        ps_s[:, j * 512:(j + 1) * 512],
                        lhsT=sb_KT[rg:rg + 8, ki * 128:(ki + 1) * 128],
                        rhs=sb_QT[rg:rg + 8, qsl], start=True, stop=True,
                        tile_position=(rg, 0))
                ps_s_t[g] = ps_s

            def emit_sig_out(g):
                qc, kg = divmod(g, NG)
                ps_s = ps_s_t[g]
                ps_s_t[g] = None
                probs = work.tile([128, KG * 512], BF16, tag="probs")
                nc.scalar.activation(probs, ps_s, ACTF.Sigmoid)
                last = qc == NQ - 1
                for j in range(KG):
                    ki = kg * KG + j
                    cg = 32 * (ki // 10) if last else 32 * (ki % 4)
                    nc.tensor.matmul(
                        ps_o[qc][cg:cg + 8, :],
                        lhsT=sb_Vkd[:, ki // 20, ki % 2, (ki % 20) // 2, :],
                        rhs=probs[:, j * 512:(j + 1) * 512],
                        start=(ki % 10 == 0 if last else ki < 4),
                        stop=(ki % 10 == 9 if last else ki >= NK - 4),
                        tile_position=(0, cg), skip_group_check=True)

            def emit_epilogue(qc):
                qsl = slice(qc * 512, (qc + 1) * 512)
                po = ps_o[qc]
                o01 = work2.tile([8, 512], F32, tag="o01")
                nc.vector.tensor_copy(o01, po[0:8, :])
                o02 = work2.tile([8, 512], F32, tag="o02")
                nc.vector.tensor_tensor(o02, po[32:40, :], o01, ALU.add)
                o03 = work2.tile([8, 512], F32, tag="o03")
                nc.vector.tensor_tensor(o03, po[64:72, :], o02, ALU.add)
                oS = work2.tile([8, 512], F32, tag="oS")
                nc.vector.tensor_tensor(oS, po[96:104, :], o03, ALU.add)
                msk = work2.tile([8, 512], F32, tag="msk")
                nc.vector.tensor_scalar(msk, oS, 0.5, None, ALU.is_gt)
                oT = work2.tile([8, 512], F32R, tag="ot")
                nc.vector.tensor_tensor(oT, oS, msk, ALU.mult)
                ps_y = ptp.tile([2, 512], F32, tag="tmp", name="ps_y")
                nc.tensor.matmul(ps_y, lhsT=W(C_FCO, C_FCO + 2, 0, 8),
                                 rhs=oT)
                nc.vector.tensor_scalar(sb_y[:, qsl], ps_y, bias(B_FCOB, 2),
                                        None, ALU.add)
                nc.sync.dma_start(y_out.ap()[:, qsl], sb_y[:, qsl])

            # fused pipeline: pairs 0-1 + chunk-0 Q path up front (DVE outers),
            # later chunks' pairs (GPSIMD outers) and Q paths emitted inside
            # the previous chunk's attention groups
            emit_pair(0, nc.vector, nc.scalar, pop)
            emit_pair(1, nc.vector, nc.scalar, pop)
            emit_oh(4, 20)
            emit_lcx2q(0)
            stg = {}
            emit_scores(0)
            for g in range(TOT):
                if g + 1 < TOT:
                    emit_scores(g + 1)
                emit_sig_out(g)
                qc, kg = divmod(g, NG)
                if qc + 1 < NQ:
                    if kg == 1:
                        emit_pair(2 * (qc + 1), nc.vector)
                    elif kg == 2:
                        emit_pair(2 * (qc + 1) + 1, nc.vector)
                    elif kg == 3:
                        emit_lcx2q(qc + 1)
                if qc == 0 and kg == 3:
                    emit_kt(0, 32, nc.sync)
                elif qc == 0 and kg == 5:
                    emit_kt(1, 0, nc.sync)
                elif qc == 0 and kg == 6:
                    emit_kt(1, 32, nc.sync)
                if kg == 8 and qc > 0:
                    emit_epilogue(qc - 1)
                if qc == NQ - 1:
                    po = ps_o[qc]
                    if kg == 6:
                        o01s = work2.tile([8, 512], F32, tag="o01",
                                          name="o01s")
                        nc.vector.tensor_copy(o01s, po[0:8, :])
                        stg[0] = o01s
                    elif kg == 11:
                        o02s = work2.tile([8, 512], F32, tag="o02",
                                          name="o02s")
                        nc.vector.tensor_tensor(o02s, po[32:40, :], stg[0],
                                                ALU.add)
                        stg[1] = o02s
                    elif kg == 16:
                        o03s = work2.tile([8, 512], F32, tag="o03",
                                          name="o03s")
                        nc.vector.tensor_tensor(o03s, po[64:72, :], stg[1],
                                                ALU.add)
                        stg[2] = o03s
            # final-chunk epilogue: only the last partial remains to fold in
            qsl = slice((NQ - 1) * 512, NQ * 512)
            oS = work2.tile([8, 512], F32, tag="oS")
            nc.vector.tensor_tensor(oS, ps_o[NQ - 1][96:104, :], stg[2],
                                    ALU.add)
            msk = work2.tile([8, 512], F32, tag="msk")
            nc.vector.tensor_scalar(msk, oS, 0.5, None, ALU.is_gt)
            oT = work2.tile([8, 512], F32R, tag="ot")
            nc.vector.tensor_tensor(oT, oS, msk, ALU.mult)
            ps_y = ptp.tile([2, 512], F32, tag="tmp", name="ps_y")
            nc.tensor.matmul(ps_y, lhsT=W(C_FCO, C_FCO + 2, 0, 8), rhs=oT)
            nc.vector.tensor_scalar(sb_y[:, qsl], ps_y, bias(B_FCOB, 2),
                                    None, ALU.add)
            nc.sync.dma_start(y_out.ap()[:, qsl], sb_y[:, qsl])
            if DEBUG:
                for nm, t in (("dbg_KT", sb_KT[0:8, :]), ("dbg_QT", sb_QT[0:8, :]),
                              ("dbg_lc", sb_lc), ("dbg_X", sb_X),
                              ("dbg_XT", sb_XT), ("dbg_X2", sb_X2),
                              ("dbg_cm", sb_cm), ("dbg_Vkd", sb_Vkd)):
                    dt_o = nc.dram_tensor(nm, list(t.shape), t.dtype,
                                          kind="ExternalOutput")
                    nc.sync.dma_start(dt_o.ap(), t)

    nc.compile()
    return nc


def _prep_inputs(x, metadata, w_ih, b_ih, b_hh, comp_w, comp_b, vf_w, vf_b,
                 fc_w, fc_b, fc2_w, fc2_b, fc3_w, fc3_b, fco_w, fco_b):
    f = np.float32
    bf = mybir.dt.np(BF16)
    pos = np.arange(T, dtype=f)
    pe = np.stack([np.sin(pos), np.cos(pos)], axis=-1).astype(f)  # (T,2)
    w_ih = np.asarray(w_ih, f)
    bb = np.asarray(b_ih, f) + np.asarray(b_hh, f)
    w_i, w_g, w_o = w_ih[0:4], w_ih[8:12], w_ih[12:16]
    gb_i = (pe @ w_i.T + bb[0:4]).T          # (4, T)
    gb_g = (pe @ w_g.T + bb[8:12]).T
    gb_o = (pe @ w_o.T + bb[12:16]).T
    fc2_w = np.asarray(fc2_w, f)
    fc3_w = np.asarray(fc3_w, f)
    kb = (pe @ fc2_w.T + np.asarray(fc2_b, f)).T   # (8, T)
    vb = (pe @ fc3_w.T + np.asarray(fc3_b, f)).T

    # bf16 weight blob
    csth = np.zeros((64, CSTH_W), f)
    for tl in range(10):
        r0, r1 = 2 * tl, 32 + 2 * tl
        cK, cV = C_KW + 8 * tl, C_VW + 8 * tl
        for c in range(2):
            csth[r0 + c, cK:cK + 8] = fc2_w[:, c]
            csth[r1 + c, cK:cK + 8] = fc2_w[:, c]
            csth[r0 + c, cV:cV + 8] = fc3_w[:, c]
            csth[r1 + c, cV:cV + 8] = fc3_w[:, c]
            csth[r0 + c, C_GWI + 4 * tl:C_GWI + 4 * tl + 4] = w_i[:, c]
            csth[r0 + c, C_GWO + 4 * tl:C_GWO + 4 * tl + 4] = w_o[:, c]
            csth[r0 + c, C_GG + 4 * tl:C_GG + 4 * tl + 4] = w_g[:, c]
    vf_w = np.asarray(vf_w, f)
    csth[0:4, C_VFX:C_VFLC] = vf_w[:, 0:4].T
    csth[0:32, C_VFLC:C_FC] = vf_w[:, 4:36].T
    csth[0:4, C_FC:C_FCO] = np.asarray(fc_w, f).T
    csth[0:8, C_FCO:C_FCO + 2] = np.asarray(fco_w, f).T
    csth = np.ascontiguousarray(csth)

    cwh = np.ascontiguousarray(
        np.asarray(comp_w, f).T.reshape(16, 128, CC).transpose(1, 0, 2))

    in_maps = []
    xf = np.asarray(x, f)
    mdf = np.asarray(metadata, f)
    for core in range(8):
        b_, hi = core // 2, core % 2
        xb = xf[b_]                       # (2, T, N)
        xpk = np.zeros((64, 256), f)
        for t in range(10):
            xpk[2 * t:2 * t + 2, :] = xb[:, t, :]
            xpk[32 + 2 * t:32 + 2 * t + 2, :] = xb[:, 10 + t, :]
        xg = np.zeros((20, 256), f)
        for tl in range(10):
            xg[2 * tl:2 * tl + 2, :] = xb[:, 10 * hi + tl, :]
        xh = np.ascontiguousarray(xb[:, 10 * hi:10 * hi + 10, :]).reshape(2, HL)
        xpm = np.ascontiguousarray(
            xh.reshape(2, 20, 128).transpose(2, 1, 0))     # (128, 20, 2)

        bia = np.zeros((80, BIA_W), f)
        for tl in range(10):
            bia[8 * tl:8 * tl + 8, B_KB0] = kb[:, tl]
            bia[8 * tl:8 * tl + 8, B_KB1] = kb[:, 10 + tl]
            bia[8 * tl:8 * tl + 8, B_VB0] = vb[:, tl]
            bia[8 * tl:8 * tl + 8, B_VB1] = vb[:, 10 + tl]
            t = 10 * hi + tl
            bia[4 * tl:4 * tl + 4, B_GBI] = gb_i[:, t]
            bia[4 * tl:4 * tl + 4, B_GBO] = gb_o[:, t]
            bia[4 * tl:4 * tl + 4, B_GBG] = gb_g[:, t]
        bia[0:8, B_FCB] = np.asarray(fc_b, f)
        bia[0:4, B_VFB] = np.asarray(vf_b, f)
        bia[0:2, B_FCOB] = np.asarray(fco_b, f)
        bia[0:CC, B_CMPB] = np.asarray(comp_b, f)

        mdh = np.ascontiguousarray(
            mdf[b_].reshape(CMAP, 256).reshape(16, 128, 256).transpose(1, 0, 2))

        in_maps.append(dict(
            xpk=xpk, xg=xg, xpm=xpm,
            csth=csth, bia=bia, mdh=mdh, cwh=cwh))
    return in_maps


def kernel(**inputs):
    global _nc_cache
    if _nc_cache is None:
        _nc_cache = _build()
    in_maps = _prep_inputs(**inputs)
    res = run_bass_kernel_spmd(_nc_cache, in_maps, core_ids=list(range(8)))
    out = np.zeros((B, 2, T, N), np.float32)
    for core in range(8):
        b_, hi = core // 2, core % 2
        y = np.asarray(res.results[core]["y"]).reshape(2, HT, N)
        out[b_, :, hi * HT:(hi + 1) * HT, :] = y
    return out

